# revision 1
# baseline (speedup 1.0000x reference)
"""Contrastive-learning loss kernel for Trainium2 (8 NeuronCores, Bass/Tile).

Problem (hardcoded shapes): B=16, L=512, DIN1=256, DIN2=192, DH=256, DF=128.
  emb1 = MLP_a(feature1); emb2 = MLP_b(feature2)          # (B, L, DF)
  positive = rowdot(f1, f2) + band-mean terms              # (N,)  N = B*L = 8192
  negative = logsumexp(f1 @ f2.T, axis=-1) - log N         # (N,)
  loss = mean(-positive + negative)

Sharding: data-parallel over B for embeddings/positives (2 batches per core);
the N x N negatives matrix is sharded row-wise. Each core computes the full
emb2 from a column-ROTATED copy of feature2 (its own batches first), so the
device program is identical across cores (pure SPMD, no partition-id): the
local rows are always columns [0, 1024) and logsumexp is invariant to column
order.

Dataflow on each core:
  x2T (bf16, 192 x 8192) -> h2T = relu(W1b^T @ x2T + b1b) (f32r)
                         -> e2T = W2b^T @ h2T + b2b (f32r, full-rate fp32)
  x1T (bf16, 256 x 1024) -> h1T -> e1T (128 x 1024)
  banded positives: W_sum = BandMask01 @ e as bf16 matmuls, then per-row dots
  and a 1/count rescale on VectorE (counts are exact host-built fp32).
  negatives: MLP2 column production is interleaved with consumption in
  1024-col groups: per (128-row tile, group), 2 f32r matmuls fill a
  (128, 1024) PSUM tile and ScalarE activation(Exp, accum_out=...) fuses
  exp + row-sum; the accumulator is fp32 so the bf16 out is just a sink.

Outputs per core: pos_out (128, 8), se_out (128, 8) where column t holds
local rows [t*128, (t+1)*128). Host: loss = mean(-pos + log(se) - log N).
"""

import numpy as np

import concourse.bacc as bacc
import concourse.tile as tile
from concourse import mybir
from concourse.bass_utils import run_bass_kernel_spmd
from concourse.masks import make_identity

F32 = mybir.dt.float32
F32R = mybir.dt.float32r
BF16 = mybir.dt.bfloat16

B, L, DIN1, DIN2, DH, DF = 16, 512, 256, 192, 256, 128
N = B * L            # 8192 total rows
NCORES = 8
NB = B // NCORES     # 2 local batches per core
NLOC = NB * L        # 1024 local rows per core
NT = NLOC // 128     # 8 local row tiles
NEG_FD = 1024        # columns exp'd per activation instruction
NGRP = N // NEG_FD   # 8 column groups


def _build(share_tgt: bool):
    nc = bacc.Bacc("TRN2", target_bir_lowering=False, debug=False)

    x1t_d = nc.dram_tensor("x1t", [DIN1, NLOC], BF16, kind="ExternalInput")
    x2t_d = nc.dram_tensor("x2t", [DIN2, N], BF16, kind="ExternalInput")
    w1a_d = nc.dram_tensor("w1a", [DIN1, DH], BF16, kind="ExternalInput")
    w2a_d = nc.dram_tensor("w2a", [DH, DF], F32R, kind="ExternalInput")
    w1b_d = nc.dram_tensor("w1b", [DIN2, DH], BF16, kind="ExternalInput")
    w2b_d = nc.dram_tensor("w2b", [DH, DF], F32R, kind="ExternalInput")
    b1a_d = nc.dram_tensor("b1a", [128, 2], F32, kind="ExternalInput")
    b2a_d = nc.dram_tensor("b2a", [128, 1], F32, kind="ExternalInput")
    b1b_d = nc.dram_tensor("b1b", [128, 2], F32, kind="ExternalInput")
    b2b_d = nc.dram_tensor("b2b", [128, 1], F32, kind="ExternalInput")
    bms_d = nc.dram_tensor("bms", [L, L], BF16, kind="ExternalInput")
    cis_d = nc.dram_tensor("cis", [128, NT], F32, kind="ExternalInput")
    if not share_tgt:
        bmt_d = nc.dram_tensor("bmt", [L, L], BF16, kind="ExternalInput")
        cit_d = nc.dram_tensor("cit", [128, NT], F32, kind="ExternalInput")
    pos_d = nc.dram_tensor("pos_out", [128, NT], F32, kind="ExternalOutput")
    se_d = nc.dram_tensor("se_out", [128, NT], F32, kind="ExternalOutput")

    with tile.TileContext(nc) as tc:
        import contextlib

        with contextlib.ExitStack() as stack:
            const = stack.enter_context(tc.tile_pool(name="const", bufs=1))
            big = stack.enter_context(tc.tile_pool(name="big", bufs=1))
            h2pool = stack.enter_context(tc.tile_pool(name="h2pool", bufs=3))
            posp = stack.enter_context(tc.tile_pool(name="posp", bufs=2))
            expp = stack.enter_context(tc.tile_pool(name="expp", bufs=3))

            # ---- constants / inputs (ordered so MLP1 can start ASAP) ----
            ident = const.tile([128, 128], F32)
            make_identity(nc, ident[:])

            w1a = const.tile([128, 2, DH], BF16)
            nc.sync.dma_start(
                out=w1a[:], in_=w1a_d.ap().rearrange("(t p) m -> p t m", p=128)
            )
            x1t = big.tile([128, 2, NLOC], BF16)
            for cc in range(2):
                nc.sync.dma_start(
                    out=x1t[:, :, cc * 512 : (cc + 1) * 512],
                    in_=x1t_d.ap().rearrange("(t p) c -> p t c", p=128)[
                        :, :, cc * 512 : (cc + 1) * 512
                    ],
                )
            w2a = const.tile([128, 2, DF], F32R)
            nc.sync.dma_start(
                out=w2a[:], in_=w2a_d.ap().rearrange("(t p) m -> p t m", p=128)
            )
            b1a = const.tile([128, 2], F32)
            nc.sync.dma_start(out=b1a[:], in_=b1a_d.ap())
            b2a = const.tile([128, 1], F32)
            nc.sync.dma_start(out=b2a[:], in_=b2a_d.ap())
            bms = const.tile([128, 4, L], BF16)
            nc.sync.dma_start(
                out=bms[:], in_=bms_d.ap().rearrange("(t p) j -> p t j", p=128)
            )
            cis = const.tile([128, NT], F32)
            nc.sync.dma_start(out=cis[:], in_=cis_d.ap())
            if share_tgt:
                bmt, cit = bms, cis
            else:
                bmt = const.tile([128, 4, L], BF16)
                nc.sync.dma_start(
                    out=bmt[:], in_=bmt_d.ap().rearrange("(t p) j -> p t j", p=128)
                )
                cit = const.tile([128, NT], F32)
                nc.sync.dma_start(out=cit[:], in_=cit_d.ap())

            w1b_a = const.tile([128, DH], BF16)
            nc.sync.dma_start(out=w1b_a[:], in_=w1b_d.ap()[0:128, :])
            w1b_b = const.tile([64, DH], BF16)
            nc.sync.dma_start(out=w1b_b[:], in_=w1b_d.ap()[128:192, :])
            w2b = const.tile([128, 2, DF], F32R)
            nc.sync.dma_start(
                out=w2b[:], in_=w2b_d.ap().rearrange("(t p) m -> p t m", p=128)
            )
            b1b = const.tile([128, 2], F32)
            nc.sync.dma_start(out=b1b[:], in_=b1b_d.ap())
            b2b = const.tile([128, 1], F32)
            nc.sync.dma_start(out=b2b[:], in_=b2b_d.ap())

            # stream feature2^T (bf16) in 2048-column chunks, local cols first
            x2a = big.tile([128, N], BF16)
            x2b = big.tile([64, N], BF16)
            for g in range(4):
                cs = slice(g * 2048, (g + 1) * 2048)
                nc.sync.dma_start(out=x2a[:, cs], in_=x2t_d.ap()[0:128, cs])
                nc.sync.dma_start(out=x2b[:, cs], in_=x2t_d.ap()[128:DIN2, cs])

            e1t = big.tile([128, NLOC], F32R)
            e2t = big.tile([128, N], F32R)
            h1t = big.tile([128, 2, NLOC], F32R)
            e1nat = big.tile([128, NT, DF], BF16)
            e2nat = big.tile([128, NT, DF], BF16)
            w1nat = big.tile([128, NT, DF], F32)
            w2snat = big.tile([128, NT, DF], F32)
            w2tnat = w2snat if share_tgt else big.tile([128, NT, DF], F32)
            pos_all = big.tile([128, NT], F32)
            acc_all = big.tile([128, NT * NGRP], F32)
            se_all = big.tile([128, NT], F32)

            psA = stack.enter_context(tc.tile_pool(name="psumA", bufs=1, space="PSUM"))

            # ---- PE warm-up: get HAM to 8/8 while input DMAs stream ----
            zr_l = const.tile([128, 128], BF16)
            nc.gpsimd.memset(zr_l[:], 0.0)
            zr_r = const.tile([128, 512], BF16)
            nc.gpsimd.memset(zr_r[:], 0.0)
            warm_ps = psA.tile([128, 512], F32, tag="sps", bufs=2)
            for _ in range(8):
                nc.tensor.matmul(warm_ps[:], zr_l[:], zr_r[:], start=True, stop=True)

            # ---- MLP1: h1T = relu(W1a^T @ x1T + b1a); e1T = W2a^T @ h1T + b2a
            for cc in range(2):
                cols = slice(cc * 512, (cc + 1) * 512)
                h1ps = psA.tile([128, 2, 512], F32, tag="hps", bufs=1)
                for mt in range(2):
                    for kt in range(2):
                        nc.tensor.matmul(
                            h1ps[:, mt, :],
                            w1a[:, kt, mt * 128 : (mt + 1) * 128],
                            x1t[:, kt, cols],
                            start=(kt == 0),
                            stop=(kt == 1),
                        )
                for mt in range(2):
                    nc.vector.tensor_scalar(
                        out=h1t[:, mt, cols],
                        in0=h1ps[:, mt, :],
                        scalar1=b1a[:, mt : mt + 1],
                        scalar2=0.0,
                        op0=mybir.AluOpType.add,
                        op1=mybir.AluOpType.max,
                    )
                e1ps = psA.tile([128, 512], F32, tag="sps", bufs=2)
                for kt in range(2):
                    nc.tensor.matmul(
                        e1ps[:],
                        w2a[:, kt, :],
                        h1t[:, kt, cols],
                        start=(kt == 0),
                        stop=(kt == 1),
                    )
                nc.vector.tensor_scalar_add(out=e1t[:, cols], in0=e1ps[:], scalar1=b2a[:])

            def transpose_to(dst, srcT, t):
                tp = psA.tile([128, 128], F32, tag="sps", bufs=2, name=f"tp{t}")
                nc.tensor.transpose(
                    tp[:], srcT[:, t * 128 : (t + 1) * 128].bitcast(F32), ident[:]
                )
                nc.vector.tensor_copy(dst[:, t, :], tp[:])

            # banded sums: W_sum[j,:] = sum_{|m-j|<=r} e[m,:]  (bf16 matmuls)
            def band(dst, bm, src):
                for b in range(NB):
                    for jt in range(4):
                        wps = psA.tile([128, 128], F32, tag="sps", bufs=2)
                        for mt in range(4):
                            nc.tensor.matmul(
                                wps[:],
                                bm[:, mt, jt * 128 : (jt + 1) * 128],
                                src[:, 4 * b + mt, :],
                                start=(mt == 0),
                                stop=(mt == 3),
                            )
                        nc.vector.tensor_copy(dst[:, 4 * b + jt, :], wps[:])

            # e1-side work only needs MLP1 -> runs while x2 still streams in
            for t in range(NT):
                transpose_to(e1nat, e1t, t)
            band(w1nat, bms, e1nat)

            # ---- MLP2 over all N tokens ----
            def mlp2_chunk(ct):
                cols = slice(ct * 512, (ct + 1) * 512)
                h2ps = psA.tile([128, 2, 512], F32, tag="hps", bufs=1, name=f"h2ps{ct}")
                for mt in range(2):
                    msl = slice(mt * 128, (mt + 1) * 128)
                    nc.tensor.matmul(
                        h2ps[:, mt, :], w1b_a[:, msl], x2a[:, cols], start=True, stop=False
                    )
                    nc.tensor.matmul(
                        h2ps[:, mt, :], w1b_b[:, msl], x2b[:, cols], start=False, stop=True
                    )
                h2t = h2pool.tile([128, 2, 512], F32R, tag="h2t", name=f"h2t{ct}")
                for mt in range(2):
                    nc.vector.tensor_scalar(
                        out=h2t[:, mt, :],
                        in0=h2ps[:, mt, :],
                        scalar1=b1b[:, mt : mt + 1],
                        scalar2=0.0,
                        op0=mybir.AluOpType.add,
                        op1=mybir.AluOpType.max,
                    )
                e2ps = psA.tile([128, 512], F32, tag="sps", bufs=2, name=f"e2ps{ct}")
                for kt in range(2):
                    nc.tensor.matmul(
                        e2ps[:], w2b[:, kt, :], h2t[:, kt, :], start=(kt == 0), stop=(kt == 1)
                    )
                nc.vector.tensor_scalar_add(out=e2t[:, cols], in0=e2ps[:], scalar1=b2b[:])

            # ---- interleaved MLP2 + negatives, one 1024-col group at a time
            for g in range(NGRP):
                mlp2_chunk(2 * g)
                mlp2_chunk(2 * g + 1)
                if g == 0:
                    # e2-side transposes + bands (local cols = group 0)
                    for t in range(NT):
                        transpose_to(e2nat, e2t, t)
                    band(w2snat, bms, e2nat)
                    if not share_tgt:
                        band(w2tnat, bmt, e2nat)
                for t in range(NT):
                    lhs = e1t[:, t * 128 : (t + 1) * 128]
                    np_ps = psA.tile([128, NEG_FD], F32, tag="neg", bufs=2)
                    for i in range(NEG_FD // 512):
                        c0 = g * NEG_FD + i * 512
                        nc.tensor.matmul(
                            np_ps[:, i * 512 : (i + 1) * 512],
                            lhs,
                            e2t[:, c0 : c0 + 512],
                            start=True,
                            stop=True,
                        )
                    idx = t * NGRP + g
                    nc.scalar.activation(
                        out=np_ps[:],
                        in_=np_ps[:],
                        func=mybir.ActivationFunctionType.Exp,
                        accum_out=acc_all[:, idx : idx + 1],
                    )

            # pos = rowdot(e1,e2loc)
            #     + (rowdot(e1,W1sum) + rowdot(e2,W2sum_s)) / cnt_s
            #     + rowdot(e1,W2sum_t) / cnt_t
            for b in range(NB):
                bsl = slice(4 * b, 4 * b + 4)
                ga = posp.tile([128, 4, DF], F32, tag="posg")
                r1 = posp.tile([128, 4], F32, tag="post")
                r2 = posp.tile([128, 4], F32, tag="post")
                if share_tgt:
                    # (W1sum + W2sum) . e1
                    nc.vector.tensor_add(ga[:], w1nat[:, bsl, :], w2snat[:, bsl, :])
                    nc.vector.tensor_mul(ga[:], ga[:], e1nat[:, bsl, :])
                else:
                    nc.vector.tensor_mul(ga[:], w1nat[:, bsl, :], e1nat[:, bsl, :])
                nc.vector.tensor_reduce(
                    out=r1[:], in_=ga[:], axis=mybir.AxisListType.X, op=mybir.AluOpType.add
                )
                gb = posp.tile([128, 4, DF], F32, tag="posg")
                nc.vector.tensor_mul(gb[:], w2snat[:, bsl, :], e2nat[:, bsl, :])
                nc.vector.tensor_reduce(
                    out=r2[:], in_=gb[:], axis=mybir.AxisListType.X, op=mybir.AluOpType.add
                )
                nc.vector.tensor_add(r1[:], r1[:], r2[:])
                nc.vector.tensor_mul(r1[:], r1[:], cis[:, bsl])
                if not share_tgt:
                    gc = posp.tile([128, 4, DF], F32, tag="posg")
                    nc.vector.tensor_mul(gc[:], w2tnat[:, bsl, :], e1nat[:, bsl, :])
                    rt = posp.tile([128, 4], F32, tag="post")
                    nc.vector.tensor_reduce(
                        out=rt[:], in_=gc[:], axis=mybir.AxisListType.X,
                        op=mybir.AluOpType.add,
                    )
                    nc.vector.tensor_mul(rt[:], rt[:], cit[:, bsl])
                    nc.vector.tensor_add(r1[:], r1[:], rt[:])
                gd = posp.tile([128, 4, DF], BF16, tag="posgb")
                nc.vector.tensor_mul(gd[:], e1nat[:, bsl, :], e2nat[:, bsl, :])
                r3 = posp.tile([128, 4], F32, tag="post")
                nc.vector.tensor_reduce(
                    out=r3[:], in_=gd[:], axis=mybir.AxisListType.X, op=mybir.AluOpType.add
                )
                nc.vector.tensor_add(pos_all[:, bsl], r1[:], r3[:])
            nc.sync.dma_start(out=pos_d.ap(), in_=pos_all[:])

            nc.vector.tensor_reduce(
                out=se_all[:],
                in_=acc_all[:].rearrange("p (t g) -> p t g", t=NT),
                axis=mybir.AxisListType.X,
                op=mybir.AluOpType.add,
            )
            nc.sync.dma_start(out=se_d.ap(), in_=se_all[:])

    nc.compile()
    return nc


_BUILD_CACHE: dict = {}


def _get_nc(share_tgt: bool):
    if share_tgt not in _BUILD_CACHE:
        _BUILD_CACHE[share_tgt] = _build(share_tgt)
    return _BUILD_CACHE[share_tgt]


def _band_mask(r: int) -> np.ndarray:
    """mask[m, j] = 1 if |m-j| <= r (and inside [0,L)) else 0."""
    bm = np.zeros((L, L), dtype=np.float32)
    if r > 0:
        j = np.arange(L)
        lo = np.maximum(j - r, 0)
        hi = np.minimum(j + r + 1, L)
        m = np.arange(L)[:, None]
        bm = ((m >= lo[None, :]) & (m < hi[None, :])).astype(np.float32)
    return bm


def _cnt_inv(r: int) -> np.ndarray:
    """(128, NT) tile of 1/count(j) per local row (j = row mod L)."""
    j = np.arange(L)
    if r > 0:
        cnt = (np.minimum(j + r + 1, L) - np.maximum(j - r, 0)).astype(np.float64)
    else:
        cnt = np.ones(L)
    cinv = (1.0 / cnt).astype(np.float32)
    rows = (np.arange(NLOC) % L)
    return np.ascontiguousarray(cinv[rows].reshape(NT, 128).T)


def kernel(**inputs):
    loss, _ = _run(inputs, trace=False)
    return loss


def _run(inputs, trace=False, trace_kwargs=None):
    import ml_dtypes

    bf16 = ml_dtypes.bfloat16
    feature1 = inputs["feature1"]
    feature2 = inputs["feature2"]
    W1a, b1a, W2a, b2a = inputs["W1a"], inputs["b1a"], inputs["W2a"], inputs["b2a"]
    W1b, b1b, W2b, b2b = inputs["W1b"], inputs["b1b"], inputs["W2b"], inputs["b2b"]
    f1 = np.ascontiguousarray(np.asarray(feature1, dtype=np.float32))
    f2 = np.ascontiguousarray(np.asarray(feature2, dtype=np.float32))
    r_self = int(np.asarray(inputs["positive_range_self"]))
    r_tgt = int(np.asarray(inputs["positive_range_tgt"]))
    share_tgt = r_tgt == r_self

    nc = _get_nc(share_tgt)

    x2t_full = np.ascontiguousarray(f2.reshape(N, DIN2).T.astype(bf16))  # (192, 8192)
    common = {
        "w1a": np.ascontiguousarray(np.asarray(W1a, np.float32).astype(bf16)),
        "w2a": np.ascontiguousarray(np.asarray(W2a, np.float32)),
        "w1b": np.ascontiguousarray(np.asarray(W1b, np.float32).astype(bf16)),
        "w2b": np.ascontiguousarray(np.asarray(W2b, np.float32)),
        "b1a": np.ascontiguousarray(np.asarray(b1a, np.float32).reshape(2, 128).T),
        "b2a": np.asarray(b2a, np.float32).reshape(128, 1),
        "b1b": np.ascontiguousarray(np.asarray(b1b, np.float32).reshape(2, 128).T),
        "b2b": np.asarray(b2b, np.float32).reshape(128, 1),
        "bms": _band_mask(r_self).astype(bf16),
        "cis": _cnt_inv(r_self),
    }
    if not share_tgt:
        common["bmt"] = _band_mask(r_tgt).astype(bf16)
        common["cit"] = _cnt_inv(r_tgt)

    in_maps = []
    for c in range(NCORES):
        x1t = np.ascontiguousarray(
            f1[c * NB : (c + 1) * NB].reshape(NLOC, DIN1).T.astype(bf16)
        )  # (256, 1024)
        # rotate feature2^T columns so this core's rows come first
        x2t = np.ascontiguousarray(
            np.concatenate(
                [x2t_full[:, c * NLOC :], x2t_full[:, : c * NLOC]], axis=1
            )
        )
        in_maps.append({**common, "x1t": x1t, "x2t": x2t})

    res = run_bass_kernel_spmd(
        nc,
        in_maps,
        core_ids=list(range(NCORES)),
        trace=trace,
        **(trace_kwargs or {}),
    )

    pos = np.empty(N, dtype=np.float64)
    se = np.empty(N, dtype=np.float64)
    for c in range(NCORES):
        # column t holds local rows [t*128, (t+1)*128) in partitions
        p = res.results[c]["pos_out"]  # (128, NT)
        s = res.results[c]["se_out"]
        pos[c * NLOC : (c + 1) * NLOC] = p.T.reshape(NLOC)
        se[c * NLOC : (c + 1) * NLOC] = s.T.reshape(NLOC)

    neg = np.log(se) - np.log(float(N))
    loss = np.mean(-pos + neg)
    return np.array(loss, dtype=np.float32), res

